# revision 127
# baseline (speedup 1.0000x reference)
"""Trainium2 Bass kernel for MultiHeadAttention (B=4, S=2048, D=1024, H=16).

Sharding: 8 cores = 4 batches x 2 sequence-halves, no collectives. Each
core computes full k/v projections for its batch and q/attention/
out-proj/LayerNorm for its half of the sequence. A host-side column
roll of x^T makes the program identical across cores (softmax over
keys is permutation-invariant): the core's q rows are always columns
[0, SQ) of its x^T.

fp8 DoubleRow with full error compensation: every value v is split as
v = vh + vl (both fp8e4, vl = fp8(v - vh), representation error ~0.05%).
A DoubleRow matmul contracts two (lhsT, rhs) 128-row slots per pass at
0.5 cycles/row, so:

  projections (q/k/v): W.x = Wh.xh + Wl.xh + Wh.xl (lo*lo dropped),
    each term pairing two k-tiles per DR pass: 12 DR passes replace 8
    bf16 matmuls per output chunk (1.33x).
  scores per head: k and q each split hi/lo; kstack = [kh;kl] on the
    partition dim, qstack = [qh;qh] / [ql;ql] duplicated. One DR matmul
    (slot0 = kstack.qdup_hi, slot1 = kstack.qdup_lo) yields the exact
    (kh+kl).(qh+ql) score at half the bf16 cost (2x).

The hi/lo stacking/duplication quadrants that cross partition halves
are built with SBUF->SBUF DMAs off the natural-layout quantization
outputs. x and all weights are split hi/lo on the host. av and the
out-projection stay bf16 (attention weights can't be fp8 without a
~2.6% error), output is written bf16 and widened on the host.

Attention per head-pair m (heads 2m/2m+1), per q-chunk n, key tile j:
  scT[h] = DR-matmul -> PSUM [128, 2, 512]
  eT     = exp(scT / 8)  one wide ScalarE op, bf16
  avT_h += v_aug_h[j].T @ eT_h[j]   [65, QC] PSUM accum (row 64 =
                                    softmax denominator via ones col)
  aoT_h  = avT_h[0:64] * (1/avT_h[64])
  out = aoT.T @ Wo.T + bo; LayerNorm -> DRAM (bf16).

Scheduling: weights stream per head-pair so attention on pair m starts
as soon as q/k(m) and the first v d-chunk exist; remaining projections
fill PE slack underneath the attention blocks.
"""

import os
import sys
from contextlib import ExitStack

for _p in ("/opt/trn_rl_repo", "/root/.axon_site/_ro/trn_rl_repo"):
    if _p not in sys.path and os.path.isdir(_p):
        sys.path.insert(0, _p)

# The kernel executes through the axon jax platform; a cpu-pinned
# JAX_PLATFORMS (used for running references) would hide the NeuronCores.
# Only safe to fix before jax is first imported.
if "jax" not in sys.modules and "axon" not in os.environ.get(
        "JAX_PLATFORMS", "axon"):
    os.environ.pop("JAX_PLATFORMS")

import ml_dtypes
import numpy as np

import concourse.bacc as bacc
import concourse.mybir as mybir
import concourse.tile as tile
from concourse import library_config
from concourse.bass_utils import run_bass_kernel_spmd

BF16 = mybir.dt.bfloat16
F32 = mybir.dt.float32
FP8 = mybir.dt.float8e4
AF = mybir.ActivationFunctionType
ALU = mybir.AluOpType
DR = mybir.MatmulPerfMode.DoubleRow

HD = 64  # head dim


def build_bass(S, SQ, D, H, dtype=BF16, qk_bias=False, ln_affine=True):
    """Build the per-core Bass program. S: kv seq len, SQ: q rows handled
    by this core, D: embed dim, H: total heads. qk_bias: emit the extra
    bias-add hop in the q/k quantization chain (host sets it False when
    bq/bk are all-zero, which shortens the attention prefetch chain).
    ln_affine: emit the LayerNorm scale/shift ops (False when ln_w is all
    ones and ln_b all zeros)."""
    assert D == H * HD
    P = 128
    ET = D // P           # e (contraction) tiles; also head-pair count
    EP = ET // 2          # k-tile pairs per DR pass
    QC = min(512, SQ)     # q free-dim chunk
    QN = SQ // QC
    KC = min(512, S)      # k-proj free-dim chunk
    KN = S // KC
    VC = min(256, D)      # v-proj d chunk
    VN = D // VC
    HPC = VC // HD        # heads per v chunk
    MPC = HPC // 2        # head-pairs per v chunk
    OC = min(256, D)      # out-proj d chunk
    ON = D // OC
    SJ = S // P           # key tiles
    TQ = SQ // P          # q row tiles

    nc = bacc.Bacc("TRN2", debug=False)

    xs = {}
    for w in ("xh", "xl"):
        xs[w] = nc.dram_tensor(w, [D, S], FP8, kind="ExternalInput").ap()
    ws = {}
    for w in ("wqh", "wql", "wkh", "wkl"):  # per head-pair: [m, p, t*d]
        ws[w] = nc.dram_tensor(w, [ET, P, ET * P], FP8,
                               kind="ExternalInput").ap()
    for w in ("wvh", "wvl"):
        ws[w] = nc.dram_tensor(w, [D, D], FP8, kind="ExternalInput").ap()
    ws["wo"] = nc.dram_tensor("wo", [D, D], dtype, kind="ExternalInput").ap()
    bs = {
        b: nc.dram_tensor(b, [D], F32, kind="ExternalInput").ap()
        for b in ("bq", "bk", "bv", "bo", "lnw", "lnb")
    }
    out = nc.dram_tensor("out", [SQ, D], BF16, kind="ExternalOutput").ap()

    with tile.TileContext(nc) as tc, ExitStack() as ctx:
        singles = ctx.enter_context(tc.tile_pool(name="singles", bufs=1))
        qkv = ctx.enter_context(tc.tile_pool(name="qkv", bufs=1))
        stk = ctx.enter_context(tc.tile_pool(name="stk", bufs=1))
        xp = ctx.enter_context(tc.tile_pool(name="xp", bufs=1))
        wqk = ctx.enter_context(tc.tile_pool(name="wqk", bufs=2))
        wvp = ctx.enter_context(tc.tile_pool(name="wvp", bufs=1))
        wop = ctx.enter_context(tc.tile_pool(name="wop", bufs=1))
        natp = ctx.enter_context(tc.tile_pool(name="natp", bufs=1))
        knatp = ctx.enter_context(tc.tile_pool(name="knatp", bufs=1))
        expp = ctx.enter_context(tc.tile_pool(name="expp", bufs=3))
        avsp = ctx.enter_context(tc.tile_pool(name="avsp", bufs=3))
        rcpp = ctx.enter_context(tc.tile_pool(name="rcpp", bufs=2))
        outp = ctx.enter_context(tc.tile_pool(name="outp", bufs=4))
        lnp = ctx.enter_context(tc.tile_pool(name="lnp", bufs=2))
        mmp = ctx.enter_context(tc.tile_pool(name="mm", bufs=2, space="PSUM"))
        scp = ctx.enter_context(tc.tile_pool(name="scp", bufs=2, space="PSUM"))
        avp = ctx.enter_context(tc.tile_pool(name="avp", bufs=2, space="PSUM"))

        nc.gpsimd.load_library(library_config.proxy)

        # kstack[:, h]: rows 0:64 = k_hi(h), rows 64:128 = k_lo(h)
        kstack = stk.tile([P, H, S], FP8, tag="kstack")
        # qstack[:, h, 0]: [qh;qh] duplicated; [:, h, 1]: [ql;ql]
        qstack = stk.tile([P, H, 2, SQ], FP8, tag="qstack")
        vt = qkv.tile([P, SJ, H, HD + 1], dtype, tag="vt")
        # aot split per q-chunk so the out-projection for chunk n only
        # depends on that chunk's attention blocks
        aots = [qkv.tile([P, ET, QC], dtype, tag=f"aot{n}", name=f"aot{n}")
                for n in range(QN)]

        def load_wqk(m):
            pre = []
            for w in ("wqh", "wql", "wkh", "wkl"):
                t = wqk.tile([P, ET, P], FP8, tag=w, name=w)
                nc.scalar.dma_start(
                    t, ws[w][m].rearrange("p (t d) -> p t d", d=P))
                pre.append(t)
            return pre

        # --- load x^T hi/lo per k-tile pair, alternating HWDGE queues
        pre0 = load_wqk(0)
        pre1 = load_wqk(1)
        xt8 = xp.tile([P, 2, ET, S], FP8, tag="xt8")
        H2 = S // 2
        qi = 0
        for h in range(2):  # first seq-half first: unblocks q/k chunk 0
            for lvl, src in ((0, xs["xh"]), (1, xs["xl"])):
                for i in range(EP):
                    eng = nc.sync if qi % 2 == 0 else nc.scalar
                    qi += 1
                    eng.dma_start(
                        xt8[:, lvl, 2 * i:2 * i + 2, h * H2:(h + 1) * H2],
                        src.rearrange("(t p) s -> p t s", p=P)
                        [:, 2 * i:2 * i + 2, h * H2:(h + 1) * H2])

        # --- constants ---
        bqk = singles.tile([P, 2 * ET], F32, tag="bqk")
        nc.sync.dma_start(bqk[:, :ET], bs["bq"].rearrange("(t p) -> p t", p=P))
        nc.sync.dma_start(bqk[:, ET:], bs["bk"].rearrange("(t p) -> p t", p=P))
        # free-dim bias rows, physically replicated across partitions
        # (compute engines can't read partition-step-0 APs)
        brow = {}
        D2 = D // 2
        tf = singles.tile([1, D2], F32, tag="browf")
        for b in ("bv", "bo", "lnw", "lnb"):
            t = singles.tile([P, D], dtype, tag=b)
            for c in range(2):
                nc.sync.dma_start(tf, bs[b][None, c * D2:(c + 1) * D2])
                nc.vector.tensor_scalar_add(t[0:1, c * D2:(c + 1) * D2],
                                            tf, 0.0)
            nc.gpsimd.partition_broadcast(t, t[0:1, :])
            brow[b] = t
        eps = singles.tile([P, 1], F32, tag="eps")
        nc.vector.memset(eps, 1e-5)
        nc.vector.memset(vt[:, :, :, HD:HD + 1], 1.0)
        ESC = 0.125 / 256.0

        def dr12(ps, lhs_of, rhs_of):
            """12-DR 3-term compensated accumulation into PSUM `ps`.
            lhs_of(lvl, i) / rhs_of(lvl, i) give the [P, 2, *] slot APs for
            hi/lo level and k-tile pair i."""
            idx = 0
            for llv, rlv in ((0, 0), (1, 0), (0, 1)):
                for i in range(EP):
                    nc.tensor.matmul(
                        ps, lhs_of(llv, i), rhs_of(rlv, i),
                        start=(idx == 0), stop=(idx == 3 * EP - 1),
                        perf_mode=DR,
                    )
                    idx += 1

        def quant8(dst, ps, bias, cols):
            """hi/lo fp8 quantization of a 2^13-scaled PSUM chunk into
            dst[:, 0/1, cols] at 2^4 scale. High priority: it gates the
            stack DMAs for the next attention block through the DVE queue."""
            ctx2 = tc.high_priority()
            ctx2.__enter__()
            if qk_bias:
                # fold the (2^13-scaled) bias in place first
                nc.vector.tensor_scalar(ps, ps, bias, 2.0 ** -9,
                                        ALU.add, ALU.mult)
                nc.vector.tensor_scalar_add(dst[:, 0, cols], ps, 0.0)
                nc.vector.tensor_tensor(
                    dst[:, 1, cols], ps, dst[:, 0, cols], ALU.subtract)
            else:
                nc.vector.tensor_scalar_mul(dst[:, 0, cols], ps, 2.0 ** -9)
                nc.vector.scalar_tensor_tensor(
                    dst[:, 1, cols], ps, 2.0 ** -9, dst[:, 0, cols],
                    ALU.mult, ALU.subtract)
            ctx2.__exit__(None, None, None)

        # Deferred-work queue: projection sub-units are emitted one per
        # attention step (just-in-time, priority right below the step) so
        # they fill PE slack instead of forming bursts between blocks.
        # Tags: ("qk", m) — stacks for pair m; ("v", c, j) — vt chunk c,
        # key tile j.
        fillers = []

        def drain(need=None, free=0):
            # With attention steps priority-protected, early emission of
            # filler units only widens the scheduler's choices — drain
            # aggressively; only shared-buffer WAR (qnat/knat/wv) paces them.
            while fillers and (free > 0 or (need and need(fillers[0][0]))):
                if not (need and need(fillers[0][0])):
                    free -= 1
                fillers.pop(0)[1]()

        def qk_proj(m, pre=None, defer=False):
            """q and k projections for head-pair m: fp8 DR compute, then
            hi/lo quantization and the stack-building DMAs."""
            state = {"w": pre}
            qnat = natp.tile([P, 2, SQ], FP8, tag="qnat", name="qnat")
            knat = knatp.tile([P, 2, S], FP8, tag="knat", name="knat")

            def loadu():
                state["w"] = load_wqk(m)

            def qchunk(n):
                wqh, wql = state["w"][0], state["w"][1]
                ps = mmp.tile([P, 512], F32, tag="mm", name="ps")[:, :QC]
                cols = slice(n * QC, (n + 1) * QC)
                dr12(ps,
                     lambda lv, i: (wqh if lv == 0 else wql)[:, 2 * i:2 * i + 2, :],
                     lambda lv, i: xt8[:, lv, 2 * i:2 * i + 2, cols])
                quant8(qnat, ps, bqk[:, m:m + 1], cols)

            def qdma():
                for h2 in range(2):
                    h = 2 * m + h2
                    src = slice(h2 * HD, (h2 + 1) * HD)
                    nc.sync.dma_start(qstack[0:HD, h, :, :], qnat[src, :, :])
                    nc.sync.dma_start(qstack[HD:P, h, :, :], qnat[src, :, :])

            def kchunk(n):
                wkh, wkl = state["w"][2], state["w"][3]
                ps = mmp.tile([P, 512], F32, tag="mm", name="ps")[:, :KC]
                cols = slice(n * KC, (n + 1) * KC)
                dr12(ps,
                     lambda lv, i: (wkh if lv == 0 else wkl)[:, 2 * i:2 * i + 2, :],
                     lambda lv, i: xt8[:, lv, 2 * i:2 * i + 2, cols])
                quant8(knat, ps, bqk[:, ET + m:ET + m + 1], cols)
                nc.sync.dma_start(kstack[0:HD, 2 * m, cols],
                                  knat[0:HD, 0, cols])
                nc.sync.dma_start(kstack[HD:P, 2 * m, cols],
                                  knat[0:HD, 1, cols])
                nc.sync.dma_start(kstack[0:HD, 2 * m + 1, cols],
                                  knat[HD:P, 0, cols])
                nc.sync.dma_start(kstack[HD:P, 2 * m + 1, cols],
                                  knat[HD:P, 1, cols])

            units = [] if pre is not None else [loadu]
            units += [lambda n=n: qchunk(n) for n in range(QN)]
            units.append(qdma)
            units += [lambda n=n: kchunk(n) for n in range(KN)]
            if defer:
                fillers.extend((("qk", m), u) for u in units)
            else:
                for u in units:
                    u()

        def load_wv(n):
            wvn = wvp.tile([P, 2, ET, VC], FP8, tag="wv", name="wvn")
            for lvl, w in ((0, "wvh"), (1, "wvl")):
                nc.scalar.dma_start(
                    wvn[:, lvl],
                    ws[w].rearrange("(t p) d -> p t d", p=P)
                    [:, :, n * VC:(n + 1) * VC])
            return wvn

        def v_block(n, wvn, j):
            """v projection d-chunk n, s-tile j."""
            ps = mmp.tile([P, 512], F32, tag="mm", name="ps")[:, :VC]
            cols = slice(j * P, (j + 1) * P)
            dr12(ps,
                 lambda lv, i: xt8[:, lv, 2 * i:2 * i + 2, cols],
                 lambda lv, i: wvn[:, lv, 2 * i:2 * i + 2, :])
            nc.vector.scalar_tensor_tensor(
                vt[:, j, n * HPC:(n + 1) * HPC, 0:HD],
                ps.rearrange("p (h d) -> p h d", d=HD),
                2.0 ** -13,
                brow["bv"][:, n * VC:(n + 1) * VC].rearrange(
                    "p (h d) -> p h d", d=HD),
                ALU.mult, ALU.add,
            )

        def v_proj_defer(c):
            """Push vt chunk c (weight load + per-j blocks) onto fillers."""
            state = {}

            def load(c=c):
                state["wv"] = load_wv(c)

            fillers.append((("v", c, -1), load))
            for j in range(SJ):
                fillers.append(
                    (("v", c, j), lambda j=j: v_block(c, state["wv"], j)))

        def att_exp(m, n, j):
            """DR score pair + exp for (head pair m, q-chunk n, k-tile j)."""
            sc = scp.tile([P, 2, 512], F32, tag="sc", name="sc")
            for h2 in range(2):
                lhsT = kstack[:, 2 * m + h2, j * P:(j + 1) * P] \
                    .unsqueeze(1).broadcast_to([P, 2, P])
                nc.tensor.matmul(
                    sc[:, h2, :QC], lhsT,
                    qstack[:, 2 * m + h2, :, n * QC:(n + 1) * QC],
                    perf_mode=DR,
                )
            et = expp.tile([P, 2, 512], dtype, tag="exp", name="et")
            nc.scalar.activation(et[:, :, :QC], sc[:, :, :QC], AF.Exp,
                                 scale=ESC)
            return et[:, 0, :QC], et[:, 1, :QC]

        def att_av(m, j, es, ava, avb, start, stop):
            nc.tensor.matmul(
                ava, vt[:, j, 2 * m, :], es[0], start=start, stop=stop)
            nc.tensor.matmul(
                avb, vt[:, j, 2 * m + 1, :], es[1], start=start, stop=stop)

        def att_norm(m, n, copies):
            for avs, po in zip(copies, (0, HD)):
                # bf16 reciprocal halves the tile so two fit in the old
                # footprint, unpipelining the per-block double-norm chain
                rcp = rcpp.tile([HD, 512], dtype, tag="rcp",
                                name="rcp")[:, :QC]
                with nc.allow_low_precision(reason="softmax denom in bf16"):
                    nc.vector.reciprocal(rcp[0:1, :], avs[HD:HD + 1, :])
                nc.gpsimd.partition_broadcast(rcp, rcp[0:1, :])
                nc.vector.tensor_tensor(
                    aots[n][po:po + HD, m, :],
                    avs[0:HD, :], rcp, ALU.mult,
                )

        # Offloaded-exp key tiles (computed/accumulated first so the slower
        # Pool exps never gate a block's tail; ScalarE paces the rest).
        # Pool only: the DVE queue is congested with quantization epilogues.
        JSEQ = list(range(SJ))

        def attention(m, n):
            """q-chunk n of head pair m (heads 2m, 2m+1)."""
            vc = m // MPC  # the vt chunk this pair's heads live in

            def need(tag):
                if tag[0] == "qk":
                    return tag[1] <= m
                return tag[1] < vc  # earlier v chunks must be complete

            drain(need)
            ava = avp.tile([HD + 1, 512], F32, tag="av", name="av")[:, :QC]
            avb = avp.tile([HD + 1, 512], F32, tag="av", name="av")[:, :QC]
            for idx, j in enumerate(JSEQ):
                # this step's av needs vt[(vc, j)] emitted first
                drain(lambda tag: tag[0] == "v" and tag[1] == vc
                      and tag[2] <= j, free=1)
                # sc/exp/av outrank all filler work in the scheduler so
                # projections only run in genuine PE gaps
                with tc.high_priority():
                    es = att_exp(m, n, j)
                    att_av(m, j, es, ava, avb,
                           start=(idx == 0), stop=(idx == SJ - 1))
            # quick bf16 copy releases the PSUM banks for the next block
            # (DVE: GPSIMD cannot access PSUM); the reciprocal-normalization
            # chain runs off the copies later
            copies = []
            for av in (ava, avb):
                avs = avsp.tile([HD + 1, 512], dtype, tag="avs",
                                name="avs")[:, :QC]
                nc.vector.tensor_scalar_add(avs, av, 0.0)
                copies.append(avs)
            att_norm(m, n, copies)

        def out_ln(t):
            """Out-projection + LayerNorm for q row tile t."""
            FSUB = min(512, D)
            NSUB = D // FSUB
            ot = outp.tile([P, D], dtype, tag="ot", name="ot")
            an = (t * P) // QC
            lc = t * P - an * QC
            for nn in range(ON):
                ps = mmp.tile([P, 512], F32, tag="mm", name="ps")[:, :OC]
                for k in range(ET):
                    nc.tensor.matmul(
                        ps, aots[an][:, k, lc:lc + P],
                        wo[:, k, nn * OC:(nn + 1) * OC],
                        start=(k == 0), stop=(k == ET - 1),
                    )
                nc.vector.tensor_tensor(
                    ot[:, nn * OC:(nn + 1) * OC], ps,
                    brow["bo"][:, nn * OC:(nn + 1) * OC], ALU.add)
            scr = lnp.tile([P, NSUB * 6 + 3], F32, tag="scr", name="scr")
            stats = scr[:, 0:NSUB * 6].rearrange("p (s f) -> p s f", f=6)
            mv = scr[:, NSUB * 6:NSUB * 6 + 2]
            rstd = scr[:, NSUB * 6 + 2:NSUB * 6 + 3]
            otv = ot.rearrange("p (s f) -> p s f", f=FSUB)
            for sbi in range(NSUB):
                nc.vector.bn_stats(stats[:, sbi, :], otv[:, sbi, :])
            nc.vector.bn_aggr(mv, stats)
            nc.scalar.activation(rstd, mv[:, 1:2], AF.Sqrt, bias=eps)
            nc.vector.reciprocal(rstd, rstd)
            nc.vector.tensor_scalar(
                ot, ot, mv[:, 0:1], rstd, ALU.subtract, ALU.mult)
            if ln_affine:
                # scale/shift on GpSimd: it's idle in the tail while DVE is
                # the critical path for the LN chains and PSUM epilogues
                nc.gpsimd.tensor_tensor(ot, ot, brow["lnw"], ALU.mult)
                nc.gpsimd.tensor_tensor(ot, ot, brow["lnb"], ALU.add)
            nc.sync.dma_start(
                out.rearrange("(t p) d -> p t d", p=P)[:, t, :], ot)

        # --- emission schedule ---
        # Head: q/k(0) + the first v s-tiles, then attention starts right
        # away. All n=0 attention blocks run first (interleaved with the
        # remaining projection prefetches), then the n=1 blocks — so the
        # out-projection (emitted last, lowest priority) fills PE slack
        # under the n=1 attention phase.
        wo = wop.tile([P, ET, D], dtype, tag="wo")
        qk_proj(0, pre0)
        wv0 = load_wv(0)
        v_block(0, wv0, 0)
        v_block(0, wv0, 1)
        for j in range(2, SJ):
            fillers.append((("v", 0, j), lambda j=j: v_block(0, wv0, j)))
        qk_proj(1, pre1, defer=True)
        nc.scalar.dma_start(wo, ws["wo"].rearrange("(t p) d -> p t d", p=P))
        attention(0, 0)
        for m in range(1, ET):
            if m + 1 < ET:
                qk_proj(m + 1, defer=True)
                if (m + 1) % MPC == 0 and (m + 1) // MPC < VN:
                    v_proj_defer((m + 1) // MPC)
            attention(m, 0)
        drain(lambda tag: True)
        # n>=1 attention phases, with the previous chunk's out-proj/LN
        # interleaved (its aot chunk is complete); the final chunk's tiles
        # run in the tail
        for n in range(1, QN):
            for m in range(ET):
                attention(m, n)
        for t in range(TQ):
            out_ln(t)

    nc.compile()
    return nc


# ---------------------------------------------------------------- host side

_CACHE = {}
E4 = ml_dtypes.float8_e4m3


def _get_nc(S, SQ, D, H, qk_bias=False, ln_affine=False):
    key = (S, SQ, D, H, qk_bias, ln_affine)
    if key not in _CACHE:
        _CACHE[key] = build_bass(S, SQ, D, H, qk_bias=qk_bias,
                                 ln_affine=ln_affine)
    return _CACHE[key]


def _split8(a, scale):
    """fp8 hi/lo split of a float32 array, pre-scaled into e4m3 range."""
    a = np.asarray(a, np.float32) * scale
    hi = a.astype(E4)
    lo = (a - hi.astype(np.float32)).astype(E4)
    return np.ascontiguousarray(hi), np.ascontiguousarray(lo)


def make_in_maps(x, Wq, bq, Wk, bk, Wv, bv, Wo, bo, ln_w, ln_b, n_cores=8):
    """Shard full inputs into per-core input maps (batch x seq-half)."""
    B, S, D = x.shape
    halves = n_cores // B
    SQ = S // halves
    bf = ml_dtypes.bfloat16
    ET = D // 128

    def pack_qk(W):
        # [m, p, t*128+d] = W.T[t*128+p, m*128+d]
        w4 = np.asarray(W, np.float32).T.reshape(ET, 128, ET, 128)
        return np.ascontiguousarray(
            w4.transpose(2, 1, 0, 3).reshape(ET, 128, ET * 128))

    wqh, wql = _split8(pack_qk(Wq), 512.0)
    wkh, wkl = _split8(pack_qk(Wk), 512.0)
    wvh, wvl = _split8(np.ascontiguousarray(np.asarray(Wv, np.float32).T),
                       512.0)
    # bq/bk are added in 2^13-scaled PSUM space (W x512, x x16)
    common = {
        "wqh": wqh, "wql": wql, "wkh": wkh, "wkl": wkl,
        "wvh": wvh, "wvl": wvl,
        "wo": np.ascontiguousarray(np.asarray(Wo).T).astype(bf),
        "bq": np.asarray(bq, np.float32) * 2.0 ** 13,
        "bk": np.asarray(bk, np.float32) * 2.0 ** 13,
        "bv": np.asarray(bv, np.float32), "bo": np.asarray(bo, np.float32),
        "lnw": np.asarray(ln_w, np.float32), "lnb": np.asarray(ln_b, np.float32),
    }
    in_maps = []
    for c in range(n_cores):
        b, half = c // halves, c % halves
        xTb = np.asarray(x[b], np.float32).T  # [D, S]
        if half:
            xTb = np.roll(xTb, -half * SQ, axis=1)
        xh, xl = _split8(xTb, 16.0)
        in_maps.append({"xh": xh, "xl": xl, **common})
    return in_maps, SQ


def kernel(x, Wq, bq, Wk, bk, Wv, bv, Wo, bo, ln_w, ln_b, _trace=False):
    x = np.asarray(x)
    B, S, D = x.shape
    n_cores = 8
    in_maps, SQ = make_in_maps(x, Wq, bq, Wk, bk, Wv, bv, Wo, bo, ln_w, ln_b,
                               n_cores)
    qk_bias = bool(np.any(np.asarray(bq)) or np.any(np.asarray(bk)))
    ln_affine = bool(np.any(np.asarray(ln_w) != 1.0)
                     or np.any(np.asarray(ln_b)))
    nc = _get_nc(S, SQ, D, 16, qk_bias, ln_affine)
    res = run_bass_kernel_spmd(nc, in_maps, list(range(n_cores)), trace=_trace)
    out = np.empty((B, S, D), np.float32)
    halves = n_cores // B
    for c in range(n_cores):
        b, half = c // halves, c % halves
        out[b, half * SQ:(half + 1) * SQ] = res.results[c]["out"].astype(
            np.float32)
    kernel.last_result = res
    return out


if __name__ == "__main__":
    nc = build_bass(512, 256, 256, 4)
    print("built ok")
